# revision 3
# baseline (speedup 1.0000x reference)
"""BiLSTM layer (B=32, T=512, D=512, H=512) Bass/TRN2 kernel, v6.

Sharding: 8 cores = 2 directions x 4 batch-quarters (BC=8 examples/core);
backward direction = forward scan over host-reversed input. Weights
replicated per direction.

Step-latency design (the metric is 512 x the serial h->h chain):
- Per-STEP PSUM gate tile P[t] = [128, gate(4), j(4), b(8)]; the input
  projection W_ih.x + bias + mask-bias is matmul'd into it ~8 steps ahead
  (accumulation group left open), the recurrent W_hh.h matmuls accumulate
  onto it, and the sigmoids read it directly. All slices contiguous so the
  subtile dependency tracker never falls back to whole-tile (false WARs).
- Gate order [g, f, i, o]; o-gate matmuls issue after sigmoid(g,f,i) so the
  sigmoid's PE semaphore threshold excludes them.
- Single sigmoid for g/f/i (g-tanh as 2*sigmoid(2z)-1, 2z folded into the
  weights host-side), separate sigmoid for o off the critical path.
- Chain: PE(Whh) -> Act(sig_gfi) -> DVE(fc, u, c) -> Act(tanh_c) -> DVE(h).
- Mask (ragged lengths): padded steps get f-preact += BIG, i-preact -= BIG
  via an augmented [ones; maskbias] matmul, freezing c exactly; fwd
  padded-tail h is replaced on the host by h[len-1]; bwd padded prefix
  yields h ~ 0.
"""

import os
import sys

import numpy as np

sys.path.insert(0, "/opt/trn_rl_repo")

import concourse.bass as bass  # noqa: E402
import concourse.bacc as bacc  # noqa: E402
import concourse.tile as tile  # noqa: E402
from concourse import mybir  # noqa: E402

F32 = mybir.dt.float32
F16 = mybir.dt.float16
F8 = mybir.dt.float8e4
F8_NP = mybir.dt.np(F8)
DR = mybir.MatmulPerfMode.DoubleRow
AF = mybir.ActivationFunctionType
ALU = mybir.AluOpType

B, D, H = 32, 512, 512
G = 4 * H
NCORES = 8
BC = 8  # batch per core
KT = D // 128  # 4 k-tiles
SX = 8  # steps per x-window DMA
LEAD = 6  # steps of prepass lead (PSUM tiles are bank-granular: 8 banks)
BIG = 60.0

_T_DEFAULT = 512


def _build_nc(T: int):
    nwin = T // SX
    nc = bacc.Bacc("TRN2", target_bir_lowering=False, debug=False, num_devices=NCORES)

    xT_d = nc.dram_tensor("xT", [D, T * BC], F16, kind="ExternalInput")
    wih_d = nc.dram_tensor("wih", [D, G], F16, kind="ExternalInput")
    whh_d = nc.dram_tensor("whh", [H, G], F8, kind="ExternalInput")
    augw_d = nc.dram_tensor("augw", [2, G], F16, kind="ExternalInput")
    aug_d = nc.dram_tensor("aug", [2, T * BC], F16, kind="ExternalInput")
    hout_d = nc.dram_tensor("hout", [T, 128, KT * BC], F16, kind="ExternalOutput")

    with tile.TileContext(nc) as tc:
        with (
            tc.tile_pool(name="const", bufs=1) as constp,
            tc.tile_pool(name="xc", bufs=2) as xcp,
            tc.tile_pool(name="hst", bufs=2) as hsp,
            tc.tile_pool(name="cst", bufs=2) as cp,
            tc.tile_pool(name="sgp", bufs=2) as sgp,
            tc.tile_pool(name="sop", bufs=2) as sop,
            tc.tile_pool(name="fcp", bufs=2) as fcp,
            tc.tile_pool(name="up", bufs=2) as up,
            tc.tile_pool(name="tcp", bufs=2) as tcp,
            tc.tile_pool(name="h8p", bufs=2) as h8p,
            tc.tile_pool(name="gpsum", bufs=8, space="PSUM") as gp,
        ):
            # ---- persistent weights in SBUF ----
            wih_sb = constp.tile([128, KT, G], F16, tag="wih")
            whh_sb = constp.tile([128, 2, 2, G], F8, tag="whh")
            augw_sb = constp.tile([2, G], F16, tag="augw")
            aug_sb = constp.tile([2, T * BC], F16, tag="aug")
            for k in range(KT):
                nc.sync.dma_start(wih_sb[:, k, :], wih_d[k * 128:(k + 1) * 128, :])
                nc.sync.dma_start(
                    whh_sb[:, k // 2, k % 2, :], whh_d[k * 128:(k + 1) * 128, :]
                )
            nc.sync.dma_start(augw_sb[:], augw_d[:])
            nc.sync.dma_start(aug_sb[:], aug_d[:])

            # ---- initial state ----
            h0 = constp.tile([128, 2, 2, BC], F8, tag="h0")
            c0 = constp.tile([128, KT * BC], F32, tag="c0")
            nc.vector.memset(h0[:], 0.0)
            nc.vector.memset(c0[:], 0.0)

            xc_t = {}
            P_t = {}

            def xc_dma(w):
                """DMA the x window for window w into SBUF."""
                xc = xcp.tile([128, KT, SX * BC], F16, tag="xc")
                xc_t[w] = xc
                c0_ = w * SX * BC
                for k in range(KT):
                    nc.sync.dma_start(
                        xc[:, k, :], xT_d[k * 128:(k + 1) * 128, c0_:c0_ + SX * BC]
                    )

            def prepass_step(t):
                """Prepass matmuls for step t: W_ih.x + bias + mask into P[t].

                All accumulation groups stay OPEN (no stop): the recurrent
                Whh matmuls close them.
                """
                P = gp.tile([128, 4, 4, BC], F32, tag="P", name="P")
                P_t[t] = P
                w, s = divmod(t, SX)
                xc = xc_t[w]
                for g_ in range(16):
                    col = g_ * 128
                    out = P[:, g_ // 4, g_ % 4, :]
                    for k in range(KT):
                        # start=True on the tile's very first matmul marks the
                        # whole 2KB PSUM bank pending-zero; every group's
                        # first write then claims (replaces) its own region,
                        # later writes accumulate. No other start bits!
                        nc.tensor.matmul(
                            out,
                            wih_sb[:, k, col:col + 128],
                            xc[:, k, s * BC:(s + 1) * BC],
                            start=(g_ == 0 and k == 0),
                            stop=False,
                            skip_group_check=True,
                        )
                    nc.tensor.matmul(
                        out,
                        augw_sb[:, col:col + 128],
                        aug_sb[:, t * BC:(t + 1) * BC],
                        start=False,
                        stop=False,
                        skip_group_check=True,
                    )

            # ---- prologue ----
            xc_dma(0)
            xc_dma(1)
            for t in range(LEAD):
                prepass_step(t)

            h_prev = h0
            c_prev = c0
            hstage = {}

            for t in range(T):
                w, s = divmod(t, SX)
                P = P_t[t]
                if s == 0:
                    hstage[w] = hsp.tile(
                        [128, SX, KT * BC], F16, tag="hst", name="hst"
                    )
                    if w + 2 < nwin:
                        xc_dma(w + 2)
                    if w >= 1:
                        nc.sync.dma_start(
                            hout_d[(w - 1) * SX:w * SX].rearrange("s p c -> p s c"),
                            hstage[w - 1][:],
                        )
                        del hstage[w - 1]

                # recurrent matmuls (fp8 DoubleRow: 2 k-tiles per issue);
                # gate order g(0), f(1), i(2), o(3); o first so sigma_gfi's
                # reads never interleave with pending P writes.
                def rec_mm(gam, kp):
                    col = gam * 128
                    ga, j = divmod(gam, 4)
                    nc.tensor.matmul(
                        P[:, ga, j, :],
                        whh_sb[:, kp, :, col:col + 128],
                        h_prev[:, kp, :, :],
                        start=False,
                        stop=(kp == 1),
                        perf_mode=DR,
                        skip_group_check=True,
                    )

                # kp-major so the kp=0 half starts as soon as h8's first
                # half-write lands; o-gate first within each half.
                for kp in range(2):
                    for gam in list(range(12, 16)) + list(range(12)):
                        rec_mm(gam, kp)

                # g-gate tanh realized as 2*sigmoid(2z)-1 (2z folded into
                # weights on the host)
                sg = sgp.tile([128, 3, KT, BC], F32, tag="sg")
                nc.scalar.activation(sg[:], P[:, 0:3, :, :], AF.Sigmoid)

                so = sop.tile([128, KT * BC], F32, tag="so")
                nc.scalar.activation(
                    so[:].rearrange("p (k b) -> p k b", b=BC), P[:, 3, :, :],
                    AF.Sigmoid,
                )

                fc = fcp.tile([128, KT * BC], F32, tag="fc")
                nc.vector.tensor_mul(
                    fc[:], sg[:, 1, :, :].rearrange("p k b -> p (k b)"), c_prev[:]
                )
                u = up.tile([128, KT * BC], F32, tag="u")
                nc.vector.scalar_tensor_tensor(
                    u[:],
                    sg[:, 0, :, :].rearrange("p k b -> p (k b)"),
                    0.5,
                    sg[:, 2, :, :].rearrange("p k b -> p (k b)"),
                    ALU.subtract,
                    ALU.mult,
                )
                c_new = cp.tile([128, KT * BC], F32, tag="c")
                nc.vector.scalar_tensor_tensor(
                    c_new[:], u[:], 2.0, fc[:], ALU.mult, ALU.add
                )
                tc_t = tcp.tile([128, KT * BC], F32, tag="tc")
                nc.scalar.activation(tc_t[:], c_new[:], AF.Tanh)
                h8 = h8p.tile([128, 2, 2, BC], F8, tag="h8")
                nc.vector.tensor_mul(
                    h8[:, 0, :, :].rearrange("p a c -> p (a c)"),
                    so[:, 0:2 * BC], tc_t[:, 0:2 * BC],
                )
                nc.vector.tensor_mul(
                    h8[:, 1, :, :].rearrange("p a c -> p (a c)"),
                    so[:, 2 * BC:], tc_t[:, 2 * BC:],
                )
                nc.vector.tensor_mul(hstage[w][:, s, :], so[:], tc_t[:])

                if t + LEAD < T:
                    prepass_step(t + LEAD)

                h_prev, c_prev = h8, c_new

            # final window flush
            nc.sync.dma_start(
                hout_d[(nwin - 1) * SX:].rearrange("s p c -> p s c"),
                hstage[nwin - 1][:],
            )

    nc.compile()
    return nc


_NC_CACHE = {}


def _get_nc(T):
    if T not in _NC_CACHE:
        _NC_CACHE[T] = _build_nc(T)
    return _NC_CACHE[T]


_RUNNER_CACHE = {}


def _get_runner(nc):
    """Compile the SPMD executable once per program; reuse across calls."""
    if id(nc) in _RUNNER_CACHE:
        return _RUNNER_CACHE[id(nc)]
    import jax
    from jax.sharding import Mesh, PartitionSpec
    from jax.experimental.shard_map import shard_map
    from concourse import bass2jax

    bass2jax.install_neuronx_cc_hook()

    partition_name = (
        nc.partition_id_tensor.name if nc.partition_id_tensor is not None else None
    )
    in_names, out_names, out_avals, zero_shapes = [], [], [], []
    for alloc in nc.m.functions[0].allocations:
        if not isinstance(alloc, mybir.MemoryLocationSet):
            continue
        name = alloc.memorylocations[0].name
        if alloc.kind == "ExternalInput":
            if name != partition_name:
                in_names.append(name)
        elif alloc.kind == "ExternalOutput":
            shape = tuple(alloc.tensor_shape)
            dtype = mybir.dt.np(alloc.dtype)
            out_names.append(name)
            out_avals.append(jax.core.ShapedArray(shape, dtype))
            zero_shapes.append((shape, dtype))
    n_params = len(in_names)
    all_in_names = in_names + out_names
    if partition_name is not None:
        all_in_names = all_in_names + [partition_name]

    def _body(*args):
        operands = list(args)
        if partition_name is not None:
            operands.append(bass2jax.partition_id_tensor())
        outs = bass2jax._bass_exec_p.bind(
            *operands,
            out_avals=tuple(out_avals),
            in_names=tuple(all_in_names),
            out_names=tuple(out_names),
            lowering_input_output_aliases=(),
            sim_require_finite=True,
            sim_require_nnan=True,
            nc=nc,
        )
        return tuple(outs)

    devices = jax.devices()[:NCORES]
    mesh = Mesh(np.asarray(devices), ("core",))
    nspecs = n_params + len(out_names)
    sharded = jax.jit(
        shard_map(
            _body,
            mesh=mesh,
            in_specs=(PartitionSpec("core"),) * nspecs,
            out_specs=(PartitionSpec("core"),) * len(out_names),
            check_rep=False,
        ),
        donate_argnums=tuple(range(n_params, nspecs)),
        keep_unused=True,
    )
    runner = (sharded, in_names, out_names, out_avals, zero_shapes)
    _RUNNER_CACHE[id(nc)] = runner
    return runner


def _run_spmd(nc, in_maps):
    sharded, in_names, out_names, out_avals, zero_shapes = _get_runner(nc)
    concat_in = [
        np.concatenate([np.asarray(in_maps[c][name]) for c in range(NCORES)], axis=0)
        for name in in_names
    ]
    concat_zeros = [
        np.zeros((NCORES * s[0], *s[1:]), dt) for (s, dt) in zero_shapes
    ]
    import time as _time

    t0 = _time.perf_counter()
    out_arrs = sharded(*concat_in, *concat_zeros)
    out_arrs = [np.asarray(a) for a in out_arrs]
    _run_spmd.last_wall_s = _time.perf_counter() - t0
    return [
        {
            name: out_arrs[i].reshape(NCORES, *out_avals[i].shape)[c]
            for i, name in enumerate(out_names)
        }
        for c in range(NCORES)
    ]


_run_spmd.last_wall_s = None

# channel permutation: reference gate order [i,f,g,o] -> kernel order [g,f,i,o]
_PERM = np.concatenate([
    np.arange(2 * H, 3 * H),  # g
    np.arange(1 * H, 2 * H),  # f
    np.arange(0 * H, 1 * H),  # i
    np.arange(3 * H, 4 * H),  # o
])


def _prep_direction_weights(W_ih, W_hh, b_ih, b_hh):
    wih = np.ascontiguousarray(W_ih[_PERM].T).astype(np.float32)  # [D, G]
    whh = np.ascontiguousarray(W_hh[_PERM].T).astype(np.float32)  # [H, G]
    bsum = (b_ih + b_hh).astype(np.float32)[_PERM]  # [G]
    # tanh(z) = 2*sigmoid(2z) - 1 for the g-gate: fold the 2z into weights
    wih[:, 0:H] *= 2.0
    whh[:, 0:H] *= 2.0
    bsum[0:H] *= 2.0
    wih = wih.astype(np.float16)
    whh = whh.astype(F8_NP)
    maskvec = np.zeros(G, np.float32)
    maskvec[H:2 * H] = 1.0   # f block: +BIG on padded steps
    maskvec[2 * H:3 * H] = -1.0  # i block: -BIG on padded steps
    augw = np.stack([bsum, maskvec]).astype(np.float16)  # [2, G]
    return wih, whh, augw


def _prep_core_inputs(x, lengths, wih, whh, augw, q, reverse, T):
    xs = x[q * BC:(q + 1) * BC, :, :]  # [BC, T, D]
    ls = lengths[q * BC:(q + 1) * BC]  # [BC]
    if reverse:
        xs = xs[:, ::-1, :]
    xT = np.ascontiguousarray(xs.transpose(2, 1, 0).reshape(D, T * BC))
    mask = (ls[None, :] > np.arange(T)[:, None]).astype(np.float32)  # [T, BC]
    if reverse:
        mask = mask[::-1]
    mb = BIG * (1.0 - mask)  # [T, BC]
    aug = np.stack([np.ones(T * BC, np.float32), mb.reshape(T * BC)])
    return {
        "xT": xT.astype(np.float16),
        "wih": wih,
        "whh": whh,
        "augw": augw,
        "aug": aug.astype(np.float16),
    }


def _assemble_direction(houts, lengths, T, reverse):
    hs = []
    for q in range(4):
        h = np.asarray(houts[q]).astype(np.float32)  # [T, 128, 32]
        h = h.reshape(T, 128, KT, BC).transpose(3, 0, 2, 1).reshape(BC, T, H)
        hs.append(h)
    h = np.concatenate(hs, axis=0)  # [B, T, H]
    if reverse:
        h = h[:, ::-1, :]
    else:
        idx = np.minimum(np.arange(T)[None, :], (lengths - 1)[:, None])  # [B, T]
        h = h[np.arange(B)[:, None], idx]
    return h


def kernel(x, lengths, W_ih_f, W_hh_f, b_ih_f, b_hh_f, W_ih_b, W_hh_b, b_ih_b, b_hh_b):
    T = x.shape[1]
    x = np.asarray(x, dtype=np.float32)
    lengths = np.asarray(lengths).astype(np.int64)

    wf = _prep_direction_weights(W_ih_f, W_hh_f, b_ih_f, b_hh_f)
    wb = _prep_direction_weights(W_ih_b, W_hh_b, b_ih_b, b_hh_b)

    in_maps = []
    for r in range(NCORES):
        reverse = r >= 4
        q = r % 4
        m = _prep_core_inputs(x, lengths, *(wb if reverse else wf), q, reverse, T)
        in_maps.append(m)

    nc = _get_nc(T)
    results = _run_spmd(nc, in_maps)
    kernel.last_wall_s = _run_spmd.last_wall_s

    h_f = _assemble_direction(
        [results[r]["hout"] for r in range(4)], lengths, T, reverse=False
    )
    h_b = _assemble_direction(
        [results[r]["hout"] for r in range(4, 8)], lengths, T, reverse=True
    )
    return np.concatenate([h_f, h_b], axis=-1).astype(np.float32)


kernel.last_exec_time_ns = None
kernel.last_wall_s = None


# revision 4
# speedup vs baseline: 1.0322x; 1.0322x over previous
"""BiLSTM layer (B=32, T=512, D=512, H=512) as a Bass/TRN2 kernel on 8
NeuronCores.

Sharding: 8 cores = 2 directions x 4 batch-quarters (BC=8 examples/core);
the backward direction is a forward scan over host-reversed input. Weights
are replicated per direction.

The metric is 512 x the serial h->h dependency chain, so the design
minimizes per-step latency:

- Per-step PSUM gate tile P[t] = [128, gate(4), j(4), b(8)]. The input
  projection W_ih.x + bias + mask-bias is matmul'd into it LEAD steps ahead
  (augmented [ones; maskbias] K=2 matmul folds bias+mask); the recurrent
  W_hh.h matmuls accumulate onto the same region, and the sigmoids read the
  PSUM directly — no per-step "add xg" op. PSUM start=True zeroes a whole
  2KB bank, so only the tile's very first matmul sets it; every group's
  first write then claims its pending-zero region and later writes
  accumulate.
- All PSUM/SBUF slices on the step path are contiguous: strided access
  patterns make the subtile dependency tracker fall back to whole-tile
  ranges, which serializes disjoint reads/writes (false WARs cost ~500ns).
- Recurrent matmuls run in fp8 (e4m3) DoubleRow perf mode: 2 k-tiles per
  instruction -> 32 Matmult+Ldweights pairs instead of 128, nearly halving
  the PE block on the chain. h is written twice: fp8 (kp-split, feeds the
  matmuls) and f16 (staged for the output DMA).
- Single sigmoid over g/f/i (g-gate tanh as 2*sigmoid(2z)-1 with 2z folded
  into the weights host-side); sigma_o is a separate op off the critical
  path (o-gate matmuls run first within each k-pair block).
- Chain per step (~1.9us): PE(Whh, DR) -> Act(sig_gfi) -> DVE(fc, u, c) ->
  Act(tanh_c) -> DVE(h8) -> PE.
- Mask (ragged lengths): padded steps get f-preact += BIG, i-preact -= BIG,
  freezing c exactly; the fwd padded tail is replaced on the host by
  h[len-1]; the bwd padded prefix yields h ~ 0.
- h is staged in SBUF for 8 steps and DMA'd out per window.
"""

import os
import sys

import numpy as np

sys.path.insert(0, "/opt/trn_rl_repo")

import concourse.bass as bass  # noqa: E402
import concourse.bacc as bacc  # noqa: E402
import concourse.tile as tile  # noqa: E402
from concourse import mybir  # noqa: E402

F32 = mybir.dt.float32
F16 = mybir.dt.float16
F8 = mybir.dt.float8e4
F8_NP = mybir.dt.np(F8)
DR = mybir.MatmulPerfMode.DoubleRow
AF = mybir.ActivationFunctionType
ALU = mybir.AluOpType

B, D, H = 32, 512, 512
G = 4 * H
NCORES = 8
BC = 8  # batch per core
KT = D // 128  # 4 k-tiles
SX = 8  # steps per x-window DMA
LEAD = 6  # steps of prepass lead (PSUM tiles are bank-granular: 8 banks)
BIG = 60.0

_T_DEFAULT = 512


def _build_nc(T: int):
    nwin = T // SX
    nc = bacc.Bacc("TRN2", target_bir_lowering=False, debug=False, num_devices=NCORES)

    xT_d = nc.dram_tensor("xT", [D, T * BC], F16, kind="ExternalInput")
    wih_d = nc.dram_tensor("wih", [D, G], F16, kind="ExternalInput")
    whh_d = nc.dram_tensor("whh", [H, G], F8, kind="ExternalInput")
    augw_d = nc.dram_tensor("augw", [2, G], F16, kind="ExternalInput")
    aug_d = nc.dram_tensor("aug", [2, T * BC], F16, kind="ExternalInput")
    hout_d = nc.dram_tensor("hout", [T, 128, KT * BC], F16, kind="ExternalOutput")

    with tile.TileContext(nc) as tc:
        with (
            tc.tile_pool(name="const", bufs=1) as constp,
            tc.tile_pool(name="xc", bufs=2) as xcp,
            tc.tile_pool(name="hst", bufs=2) as hsp,
            tc.tile_pool(name="cst", bufs=2) as cp,
            tc.tile_pool(name="sgp", bufs=2) as sgp,
            tc.tile_pool(name="sop", bufs=2) as sop,
            tc.tile_pool(name="fcp", bufs=2) as fcp,
            tc.tile_pool(name="up", bufs=2) as up,
            tc.tile_pool(name="tcp", bufs=2) as tcp,
            tc.tile_pool(name="h8p", bufs=2) as h8p,
            tc.tile_pool(name="gpsum", bufs=8, space="PSUM") as gp,
        ):
            # ---- persistent weights in SBUF ----
            wih_sb = constp.tile([128, KT, G], F16, tag="wih")
            whh_sb = constp.tile([128, 2, 2, G], F8, tag="whh")
            augw_sb = constp.tile([2, G], F16, tag="augw")
            aug_sb = constp.tile([2, T * BC], F16, tag="aug")
            for k in range(KT):
                nc.sync.dma_start(wih_sb[:, k, :], wih_d[k * 128:(k + 1) * 128, :])
                nc.sync.dma_start(
                    whh_sb[:, k // 2, k % 2, :], whh_d[k * 128:(k + 1) * 128, :]
                )
            nc.sync.dma_start(augw_sb[:], augw_d[:])
            nc.sync.dma_start(aug_sb[:], aug_d[:])

            # ---- initial state ----
            h0 = constp.tile([128, 2, 2, BC], F8, tag="h0")
            c0 = constp.tile([128, KT * BC], F32, tag="c0")
            nc.vector.memset(h0[:], 0.0)
            nc.vector.memset(c0[:], 0.0)

            xc_t = {}
            P_t = {}

            def xc_dma(w):
                """DMA the x window for window w into SBUF."""
                xc = xcp.tile([128, KT, SX * BC], F16, tag="xc")
                xc_t[w] = xc
                c0_ = w * SX * BC
                for k in range(KT):
                    nc.sync.dma_start(
                        xc[:, k, :], xT_d[k * 128:(k + 1) * 128, c0_:c0_ + SX * BC]
                    )

            def prepass_step(t):
                """Prepass matmuls for step t: W_ih.x + bias + mask into P[t].

                All accumulation groups stay OPEN (no stop): the recurrent
                Whh matmuls close them.
                """
                P = gp.tile([128, 4, 4, BC], F32, tag="P", name="P")
                P_t[t] = P
                w, s = divmod(t, SX)
                xc = xc_t[w]
                for g_ in range(16):
                    col = g_ * 128
                    out = P[:, g_ // 4, g_ % 4, :]
                    for k in range(KT):
                        # start=True on the tile's very first matmul marks the
                        # whole 2KB PSUM bank pending-zero; every group's
                        # first write then claims (replaces) its own region,
                        # later writes accumulate. No other start bits!
                        nc.tensor.matmul(
                            out,
                            wih_sb[:, k, col:col + 128],
                            xc[:, k, s * BC:(s + 1) * BC],
                            start=(g_ == 0 and k == 0),
                            stop=False,
                            skip_group_check=True,
                        )
                    nc.tensor.matmul(
                        out,
                        augw_sb[:, col:col + 128],
                        aug_sb[:, t * BC:(t + 1) * BC],
                        start=False,
                        stop=False,
                        skip_group_check=True,
                    )

            # ---- prologue ----
            xc_dma(0)
            xc_dma(1)
            for t in range(LEAD):
                prepass_step(t)

            h_prev = h0
            c_prev = c0
            hstage = {}

            for t in range(T):
                w, s = divmod(t, SX)
                P = P_t[t]
                if s == 0:
                    hstage[w] = hsp.tile(
                        [128, SX, KT * BC], F16, tag="hst", name="hst"
                    )
                    if w + 2 < nwin:
                        xc_dma(w + 2)
                    if w >= 1:
                        nc.sync.dma_start(
                            hout_d[(w - 1) * SX:w * SX].rearrange("s p c -> p s c"),
                            hstage[w - 1][:],
                        )
                        del hstage[w - 1]

                # recurrent matmuls (fp8 DoubleRow: 2 k-tiles per issue);
                # gate order g(0), f(1), i(2), o(3); o first so sigma_gfi's
                # reads never interleave with pending P writes.
                def rec_mm(gam, kp):
                    col = gam * 128
                    ga, j = divmod(gam, 4)
                    nc.tensor.matmul(
                        P[:, ga, j, :],
                        whh_sb[:, kp, :, col:col + 128],
                        h_prev[:, kp, :, :],
                        start=False,
                        stop=(kp == 1),
                        perf_mode=DR,
                        skip_group_check=True,
                    )

                # kp-major so the kp=0 half starts as soon as h8's first
                # half-write lands; o-gate first within each half.
                for kp in range(2):
                    for gam in list(range(12, 16)) + list(range(12)):
                        rec_mm(gam, kp)

                # g-gate tanh realized as 2*sigmoid(2z)-1 (2z folded into
                # weights on the host)
                sg = sgp.tile([128, 3, KT, BC], F32, tag="sg")
                nc.scalar.activation(sg[:], P[:, 0:3, :, :], AF.Sigmoid)

                so = sop.tile([128, KT * BC], F32, tag="so")
                nc.scalar.activation(
                    so[:].rearrange("p (k b) -> p k b", b=BC), P[:, 3, :, :],
                    AF.Sigmoid,
                )

                fc = fcp.tile([128, KT * BC], F32, tag="fc")
                nc.vector.tensor_mul(
                    fc[:], sg[:, 1, :, :].rearrange("p k b -> p (k b)"), c_prev[:]
                )
                u = up.tile([128, KT * BC], F32, tag="u")
                nc.vector.scalar_tensor_tensor(
                    u[:],
                    sg[:, 0, :, :].rearrange("p k b -> p (k b)"),
                    0.5,
                    sg[:, 2, :, :].rearrange("p k b -> p (k b)"),
                    ALU.subtract,
                    ALU.mult,
                )
                c_new = cp.tile([128, KT * BC], F32, tag="c")
                nc.vector.scalar_tensor_tensor(
                    c_new[:], u[:], 2.0, fc[:], ALU.mult, ALU.add
                )
                tc_t = tcp.tile([128, KT * BC], F32, tag="tc")
                nc.scalar.activation(tc_t[:], c_new[:], AF.Tanh)
                h8 = h8p.tile([128, 2, 2, BC], F8, tag="h8")
                nc.vector.tensor_mul(
                    h8[:, 0, :, :].rearrange("p a c -> p (a c)"),
                    so[:, 0:2 * BC], tc_t[:, 0:2 * BC],
                )
                nc.vector.tensor_mul(
                    h8[:, 1, :, :].rearrange("p a c -> p (a c)"),
                    so[:, 2 * BC:], tc_t[:, 2 * BC:],
                )
                nc.vector.tensor_mul(hstage[w][:, s, :], so[:], tc_t[:])

                if t + LEAD < T:
                    prepass_step(t + LEAD)

                h_prev, c_prev = h8, c_new

            # final window flush
            nc.sync.dma_start(
                hout_d[(nwin - 1) * SX:].rearrange("s p c -> p s c"),
                hstage[nwin - 1][:],
            )

    nc.compile()
    return nc


_NC_CACHE = {}


def _get_nc(T):
    if T not in _NC_CACHE:
        _NC_CACHE[T] = _build_nc(T)
    return _NC_CACHE[T]


_RUNNER_CACHE = {}


def _get_runner(nc):
    """Compile the SPMD executable once per program; reuse across calls."""
    if id(nc) in _RUNNER_CACHE:
        return _RUNNER_CACHE[id(nc)]
    import jax
    from jax.sharding import Mesh, PartitionSpec
    from jax.experimental.shard_map import shard_map
    from concourse import bass2jax

    bass2jax.install_neuronx_cc_hook()

    partition_name = (
        nc.partition_id_tensor.name if nc.partition_id_tensor is not None else None
    )
    in_names, out_names, out_avals, zero_shapes = [], [], [], []
    for alloc in nc.m.functions[0].allocations:
        if not isinstance(alloc, mybir.MemoryLocationSet):
            continue
        name = alloc.memorylocations[0].name
        if alloc.kind == "ExternalInput":
            if name != partition_name:
                in_names.append(name)
        elif alloc.kind == "ExternalOutput":
            shape = tuple(alloc.tensor_shape)
            dtype = mybir.dt.np(alloc.dtype)
            out_names.append(name)
            out_avals.append(jax.core.ShapedArray(shape, dtype))
            zero_shapes.append((shape, dtype))
    n_params = len(in_names)
    all_in_names = in_names + out_names
    if partition_name is not None:
        all_in_names = all_in_names + [partition_name]

    def _body(*args):
        operands = list(args)
        if partition_name is not None:
            operands.append(bass2jax.partition_id_tensor())
        outs = bass2jax._bass_exec_p.bind(
            *operands,
            out_avals=tuple(out_avals),
            in_names=tuple(all_in_names),
            out_names=tuple(out_names),
            lowering_input_output_aliases=(),
            sim_require_finite=True,
            sim_require_nnan=True,
            nc=nc,
        )
        return tuple(outs)

    devices = jax.devices()[:NCORES]
    mesh = Mesh(np.asarray(devices), ("core",))
    nspecs = n_params + len(out_names)
    sharded = jax.jit(
        shard_map(
            _body,
            mesh=mesh,
            in_specs=(PartitionSpec("core"),) * nspecs,
            out_specs=(PartitionSpec("core"),) * len(out_names),
            check_rep=False,
        ),
        donate_argnums=tuple(range(n_params, nspecs)),
        keep_unused=True,
    )
    runner = (sharded, in_names, out_names, out_avals, zero_shapes)
    _RUNNER_CACHE[id(nc)] = runner
    return runner


def _run_spmd(nc, in_maps):
    sharded, in_names, out_names, out_avals, zero_shapes = _get_runner(nc)
    concat_in = [
        np.concatenate([np.asarray(in_maps[c][name]) for c in range(NCORES)], axis=0)
        for name in in_names
    ]
    concat_zeros = [
        np.zeros((NCORES * s[0], *s[1:]), dt) for (s, dt) in zero_shapes
    ]
    import time as _time

    t0 = _time.perf_counter()
    out_arrs = sharded(*concat_in, *concat_zeros)
    out_arrs = [np.asarray(a) for a in out_arrs]
    _run_spmd.last_wall_s = _time.perf_counter() - t0
    return [
        {
            name: out_arrs[i].reshape(NCORES, *out_avals[i].shape)[c]
            for i, name in enumerate(out_names)
        }
        for c in range(NCORES)
    ]


_run_spmd.last_wall_s = None

# channel permutation: reference gate order [i,f,g,o] -> kernel order [g,f,i,o]
_PERM = np.concatenate([
    np.arange(2 * H, 3 * H),  # g
    np.arange(1 * H, 2 * H),  # f
    np.arange(0 * H, 1 * H),  # i
    np.arange(3 * H, 4 * H),  # o
])


def _prep_direction_weights(W_ih, W_hh, b_ih, b_hh):
    wih = np.ascontiguousarray(W_ih[_PERM].T).astype(np.float32)  # [D, G]
    whh = np.ascontiguousarray(W_hh[_PERM].T).astype(np.float32)  # [H, G]
    bsum = (b_ih + b_hh).astype(np.float32)[_PERM]  # [G]
    # tanh(z) = 2*sigmoid(2z) - 1 for the g-gate: fold the 2z into weights
    wih[:, 0:H] *= 2.0
    whh[:, 0:H] *= 2.0
    bsum[0:H] *= 2.0
    wih = wih.astype(np.float16)
    whh = whh.astype(F8_NP)
    maskvec = np.zeros(G, np.float32)
    maskvec[H:2 * H] = 1.0   # f block: +BIG on padded steps
    maskvec[2 * H:3 * H] = -1.0  # i block: -BIG on padded steps
    augw = np.stack([bsum, maskvec]).astype(np.float16)  # [2, G]
    return wih, whh, augw


def _prep_core_inputs(x, lengths, wih, whh, augw, q, reverse, T):
    xs = x[q * BC:(q + 1) * BC, :, :]  # [BC, T, D]
    ls = lengths[q * BC:(q + 1) * BC]  # [BC]
    if reverse:
        xs = xs[:, ::-1, :]
    xT = np.ascontiguousarray(xs.transpose(2, 1, 0).reshape(D, T * BC))
    mask = (ls[None, :] > np.arange(T)[:, None]).astype(np.float32)  # [T, BC]
    if reverse:
        mask = mask[::-1]
    mb = BIG * (1.0 - mask)  # [T, BC]
    aug = np.stack([np.ones(T * BC, np.float32), mb.reshape(T * BC)])
    return {
        "xT": xT.astype(np.float16),
        "wih": wih,
        "whh": whh,
        "augw": augw,
        "aug": aug.astype(np.float16),
    }


def _assemble_direction(houts, lengths, T, reverse):
    hs = []
    for q in range(4):
        h = np.asarray(houts[q]).astype(np.float32)  # [T, 128, 32]
        h = h.reshape(T, 128, KT, BC).transpose(3, 0, 2, 1).reshape(BC, T, H)
        hs.append(h)
    h = np.concatenate(hs, axis=0)  # [B, T, H]
    if reverse:
        h = h[:, ::-1, :]
    else:
        idx = np.minimum(np.arange(T)[None, :], (lengths - 1)[:, None])  # [B, T]
        h = h[np.arange(B)[:, None], idx]
    return h


def kernel(x, lengths, W_ih_f, W_hh_f, b_ih_f, b_hh_f, W_ih_b, W_hh_b, b_ih_b, b_hh_b):
    T = x.shape[1]
    x = np.asarray(x, dtype=np.float32)
    lengths = np.asarray(lengths).astype(np.int64)

    wf = _prep_direction_weights(W_ih_f, W_hh_f, b_ih_f, b_hh_f)
    wb = _prep_direction_weights(W_ih_b, W_hh_b, b_ih_b, b_hh_b)

    in_maps = []
    for r in range(NCORES):
        reverse = r >= 4
        q = r % 4
        m = _prep_core_inputs(x, lengths, *(wb if reverse else wf), q, reverse, T)
        in_maps.append(m)

    nc = _get_nc(T)
    results = _run_spmd(nc, in_maps)
    kernel.last_wall_s = _run_spmd.last_wall_s

    h_f = _assemble_direction(
        [results[r]["hout"] for r in range(4)], lengths, T, reverse=False
    )
    h_b = _assemble_direction(
        [results[r]["hout"] for r in range(4, 8)], lengths, T, reverse=True
    )
    return np.concatenate([h_f, h_b], axis=-1).astype(np.float32)


kernel.last_exec_time_ns = None
kernel.last_wall_s = None


# revision 5
# speedup vs baseline: 1.0331x; 1.0009x over previous
"""BiLSTM layer (B=32, T=512, D=512, H=512) as a Bass/TRN2 kernel on 8
NeuronCores.

Sharding: 8 cores = 2 directions x 4 batch-quarters (BC=8 examples/core);
the backward direction is a forward scan over host-reversed input. Weights
are replicated per direction.

The metric is 512 x the serial h->h dependency chain, so the design
minimizes per-step latency:

- Per-step PSUM gate tile P[t] = [128, gate(4), j(4), b(8)]. The input
  projection W_ih.x + bias + mask-bias is matmul'd into it LEAD steps ahead
  (augmented [ones; maskbias] K=2 matmul folds bias+mask); the recurrent
  W_hh.h matmuls accumulate onto the same region, and the sigmoids read the
  PSUM directly — no per-step "add xg" op. PSUM start=True zeroes a whole
  2KB bank, so only the tile's very first matmul sets it; every group's
  first write then claims its pending-zero region and later writes
  accumulate.
- All PSUM/SBUF slices on the step path are contiguous: strided access
  patterns make the subtile dependency tracker fall back to whole-tile
  ranges, which serializes disjoint reads/writes (false WARs cost ~500ns).
- Recurrent matmuls run in fp8 (e4m3) DoubleRow perf mode: 2 k-tiles per
  instruction -> 32 Matmult+Ldweights pairs instead of 128, nearly halving
  the PE block on the chain. h is written twice: fp8 (kp-split, feeds the
  matmuls) and f16 (staged for the output DMA).
- Single sigmoid over g/f/i (g-gate tanh as 2*sigmoid(2z)-1 with 2z folded
  into the weights host-side); sigma_o is a separate op off the critical
  path (o-gate matmuls run first within each k-pair block).
- Chain per step (~1.9us): PE(Whh, DR) -> Act(sig_gfi) -> DVE(fc, u, c) ->
  Act(tanh_c) -> DVE(h8) -> PE.
- Mask (ragged lengths): padded steps get f-preact += BIG, i-preact -= BIG,
  freezing c exactly; the fwd padded tail is replaced on the host by
  h[len-1]; the bwd padded prefix yields h ~ 0.
- h is staged in SBUF for 8 steps and DMA'd out per window.
"""

import os
import sys

import numpy as np

sys.path.insert(0, "/opt/trn_rl_repo")

import concourse.bass as bass  # noqa: E402
import concourse.bacc as bacc  # noqa: E402
import concourse.tile as tile  # noqa: E402
from concourse import mybir  # noqa: E402

F32 = mybir.dt.float32
F16 = mybir.dt.float16
F8 = mybir.dt.float8e4
F8_NP = mybir.dt.np(F8)
DR = mybir.MatmulPerfMode.DoubleRow
AF = mybir.ActivationFunctionType
ALU = mybir.AluOpType

B, D, H = 32, 512, 512
G = 4 * H
NCORES = 8
BC = 8  # batch per core
KT = D // 128  # 4 k-tiles
SX = 8  # steps per x-window DMA
LEAD = 6  # steps of prepass lead (PSUM tiles are bank-granular: 8 banks)
BIG = 60.0

_T_DEFAULT = 512


def _build_nc(T: int):
    nwin = T // SX
    nc = bacc.Bacc("TRN2", target_bir_lowering=False, debug=False, num_devices=NCORES)

    xT_d = nc.dram_tensor("xT", [D, T * BC], F16, kind="ExternalInput")
    wih_d = nc.dram_tensor("wih", [D, G], F16, kind="ExternalInput")
    whh_d = nc.dram_tensor("whh", [H, G], F8, kind="ExternalInput")
    augw_d = nc.dram_tensor("augw", [2, G], F16, kind="ExternalInput")
    aug_d = nc.dram_tensor("aug", [2, T * BC], F16, kind="ExternalInput")
    hout_d = nc.dram_tensor("hout", [T, 128, KT * BC], F16, kind="ExternalOutput")

    with tile.TileContext(nc) as tc:
        with (
            tc.tile_pool(name="const", bufs=1) as constp,
            tc.tile_pool(name="xc", bufs=2) as xcp,
            tc.tile_pool(name="hst", bufs=2) as hsp,
            tc.tile_pool(name="sop", bufs=2) as sop,
            tc.tile_pool(name="tcp", bufs=2) as tcp,
            tc.tile_pool(name="h8p", bufs=2) as h8p,
            tc.tile_pool(name="gpsum", bufs=8, space="PSUM") as gp,
        ):
            # ---- persistent weights in SBUF ----
            wih_sb = constp.tile([128, KT, G], F16, tag="wih")
            whh_sb = constp.tile([128, 2, 2, G], F8, tag="whh")
            augw_sb = constp.tile([2, G], F16, tag="augw")
            aug_sb = constp.tile([2, T * BC], F16, tag="aug")
            for k in range(KT):
                nc.sync.dma_start(wih_sb[:, k, :], wih_d[k * 128:(k + 1) * 128, :])
                nc.sync.dma_start(
                    whh_sb[:, k // 2, k % 2, :], whh_d[k * 128:(k + 1) * 128, :]
                )
            nc.sync.dma_start(augw_sb[:], augw_d[:])
            nc.sync.dma_start(aug_sb[:], aug_d[:])

            # ---- initial state ----
            h0 = constp.tile([128, 2, 2, BC], F8, tag="h0")
            nc.vector.memset(h0[:], 0.0)
            # interleaved sigmoid outputs: odd slots hold sigma(g,f,i); even
            # slots stay zero forever (scan reset columns read them as d0=0)
            sgx = [
                constp.tile([128, 3 * KT * BC * 2], F32, tag=f"sgx{i}",
                            name=f"sgx{i}")
                for i in range(2)
            ]
            # cell-state scan buffers D: col 2j+1 holds c[j] after a scan
            # writes [0:64]; col 2j+2 holds u[j] (written per step); the scan
            # reads d1 = D[:, 1:65]
            dbuf = [
                constp.tile([128, 66], F32, tag=f"dbuf{i}", name=f"dbuf{i}")
                for i in range(2)
            ]
            for i in range(2):
                nc.vector.memset(sgx[i][:], 0.0)
                nc.vector.memset(dbuf[i][:], 0.0)

            xc_t = {}
            P_t = {}

            def xc_dma(w):
                """DMA the x window for window w into SBUF."""
                xc = xcp.tile([128, KT, SX * BC], F16, tag="xc")
                xc_t[w] = xc
                c0_ = w * SX * BC
                for k in range(KT):
                    nc.sync.dma_start(
                        xc[:, k, :], xT_d[k * 128:(k + 1) * 128, c0_:c0_ + SX * BC]
                    )

            def prepass_step(t):
                """Prepass matmuls for step t: W_ih.x + bias + mask into P[t].

                All accumulation groups stay OPEN (no stop): the recurrent
                Whh matmuls close them.
                """
                P = gp.tile([128, 4, 4, BC], F32, tag="P", name="P")
                P_t[t] = P
                w, s = divmod(t, SX)
                xc = xc_t[w]
                for g_ in range(16):
                    col = g_ * 128
                    out = P[:, g_ // 4, g_ % 4, :]
                    for k in range(KT):
                        # start=True on the tile's very first matmul marks the
                        # whole 2KB PSUM bank pending-zero; every group's
                        # first write then claims (replaces) its own region,
                        # later writes accumulate. No other start bits!
                        nc.tensor.matmul(
                            out,
                            wih_sb[:, k, col:col + 128],
                            xc[:, k, s * BC:(s + 1) * BC],
                            start=(g_ == 0 and k == 0),
                            stop=False,
                            skip_group_check=True,
                        )
                    nc.tensor.matmul(
                        out,
                        augw_sb[:, col:col + 128],
                        aug_sb[:, t * BC:(t + 1) * BC],
                        start=False,
                        stop=False,
                        skip_group_check=True,
                    )

            # ---- prologue ----
            xc_dma(0)
            xc_dma(1)
            for t in range(LEAD):
                prepass_step(t)

            h_prev = h0
            hstage = {}

            for t in range(T):
                w, s = divmod(t, SX)
                P = P_t[t]
                if s == 0:
                    hstage[w] = hsp.tile(
                        [128, SX, KT * BC], F16, tag="hst", name="hst"
                    )
                    if w + 2 < nwin:
                        xc_dma(w + 2)
                    if w >= 1:
                        nc.sync.dma_start(
                            hout_d[(w - 1) * SX:w * SX].rearrange("s p c -> p s c"),
                            hstage[w - 1][:],
                        )
                        del hstage[w - 1]

                # recurrent matmuls (fp8 DoubleRow: 2 k-tiles per issue);
                # gate order g(0), f(1), i(2), o(3); o first so sigma_gfi's
                # reads never interleave with pending P writes.
                def rec_mm(gam, kp):
                    col = gam * 128
                    ga, j = divmod(gam, 4)
                    nc.tensor.matmul(
                        P[:, ga, j, :],
                        whh_sb[:, kp, :, col:col + 128],
                        h_prev[:, kp, :, :],
                        start=False,
                        stop=(kp == 1),
                        perf_mode=DR,
                        skip_group_check=True,
                    )

                # kp-major so the kp=0 half starts as soon as h8's first
                # half-write lands; o-gate first within each half.
                for kp in range(2):
                    for gam in list(range(12, 16)) + list(range(12)):
                        rec_mm(gam, kp)

                # sigmoid(g,f,i) into the odd slots of the interleaved
                # buffer; g-gate tanh realized as 2*sigmoid(2z)-1 (2z folded
                # into weights on the host, the *2-0.5 rebuilt below)
                sg = sgx[t % 2][:].rearrange(
                    "p (ga k b two) -> p ga k b two", ga=3, k=KT, two=2
                )
                nc.scalar.activation(sg[:, :, :, :, 1], P[:, 0:3, :, :], AF.Sigmoid)

                so = sop.tile([128, KT * BC], F32, tag="so")
                nc.scalar.activation(
                    so[:].rearrange("p (k b) -> p k b", b=BC), P[:, 3, :, :],
                    AF.Sigmoid,
                )

                # The scan tracks C' = c/2 (scale-invariant recurrence), so
                # u' = (sigma_g-0.5)*sigma_i needs no doubling and the tanh
                # applies scale=2. u'[j] goes to D col 2j+2.
                Dc, Dn = dbuf[t % 2], dbuf[(t + 1) % 2]
                up_ = Dc[:, 2:66].rearrange("p (j two) -> p j two", two=2)[:, :, 0]
                nc.vector.scalar_tensor_tensor(
                    up_,
                    sg[:, 0, :, :, 1].rearrange("p k b -> p (k b)"),
                    0.5,
                    sg[:, 2, :, :, 1].rearrange("p k b -> p (k b)"),
                    ALU.subtract,
                    ALU.mult,
                )
                # cell update in ONE scan: col 2j resets state to C'_prev[j]
                # (d0 even slots are the persistent zeros), col 2j+1 computes
                # C'[j] = sigma_f[j]*C'_prev[j] + u'[j]; out col 2j+1 lands at
                # Dn[2j+1], which the NEXT scan reads as its C'_prev[j]
                nc.vector.tensor_tensor_scan(
                    Dn[:, 0:64],
                    sg[:, 1, :, :, :].rearrange("p k b two -> p (k b two)"),
                    Dc[:, 1:65],
                    0.0,
                    ALU.mult,
                    ALU.add,
                )
                tc_t = tcp.tile([128, KT * BC], F32, tag="tc")
                nc.scalar.activation(
                    tc_t[:],
                    Dn[:, 1:65].rearrange("p (j two) -> p j two", two=2)[:, :, 0],
                    AF.Tanh,
                    scale=2.0,
                )
                h8 = h8p.tile([128, 2, 2, BC], F8, tag="h8")
                nc.vector.tensor_mul(
                    h8[:, 0, :, :].rearrange("p a c -> p (a c)"),
                    so[:, 0:2 * BC], tc_t[:, 0:2 * BC],
                )
                nc.vector.tensor_mul(
                    h8[:, 1, :, :].rearrange("p a c -> p (a c)"),
                    so[:, 2 * BC:], tc_t[:, 2 * BC:],
                )
                nc.vector.tensor_mul(hstage[w][:, s, :], so[:], tc_t[:])

                if t + LEAD < T:
                    prepass_step(t + LEAD)

                h_prev = h8

            # final window flush
            nc.sync.dma_start(
                hout_d[(nwin - 1) * SX:].rearrange("s p c -> p s c"),
                hstage[nwin - 1][:],
            )

    nc.compile()
    return nc


_NC_CACHE = {}


def _get_nc(T):
    if T not in _NC_CACHE:
        _NC_CACHE[T] = _build_nc(T)
    return _NC_CACHE[T]


_RUNNER_CACHE = {}


def _get_runner(nc):
    """Compile the SPMD executable once per program; reuse across calls."""
    if id(nc) in _RUNNER_CACHE:
        return _RUNNER_CACHE[id(nc)]
    import jax
    from jax.sharding import Mesh, PartitionSpec
    from jax.experimental.shard_map import shard_map
    from concourse import bass2jax

    bass2jax.install_neuronx_cc_hook()

    partition_name = (
        nc.partition_id_tensor.name if nc.partition_id_tensor is not None else None
    )
    in_names, out_names, out_avals, zero_shapes = [], [], [], []
    for alloc in nc.m.functions[0].allocations:
        if not isinstance(alloc, mybir.MemoryLocationSet):
            continue
        name = alloc.memorylocations[0].name
        if alloc.kind == "ExternalInput":
            if name != partition_name:
                in_names.append(name)
        elif alloc.kind == "ExternalOutput":
            shape = tuple(alloc.tensor_shape)
            dtype = mybir.dt.np(alloc.dtype)
            out_names.append(name)
            out_avals.append(jax.core.ShapedArray(shape, dtype))
            zero_shapes.append((shape, dtype))
    n_params = len(in_names)
    all_in_names = in_names + out_names
    if partition_name is not None:
        all_in_names = all_in_names + [partition_name]

    def _body(*args):
        operands = list(args)
        if partition_name is not None:
            operands.append(bass2jax.partition_id_tensor())
        outs = bass2jax._bass_exec_p.bind(
            *operands,
            out_avals=tuple(out_avals),
            in_names=tuple(all_in_names),
            out_names=tuple(out_names),
            lowering_input_output_aliases=(),
            sim_require_finite=True,
            sim_require_nnan=True,
            nc=nc,
        )
        return tuple(outs)

    devices = jax.devices()[:NCORES]
    mesh = Mesh(np.asarray(devices), ("core",))
    nspecs = n_params + len(out_names)
    sharded = jax.jit(
        shard_map(
            _body,
            mesh=mesh,
            in_specs=(PartitionSpec("core"),) * nspecs,
            out_specs=(PartitionSpec("core"),) * len(out_names),
            check_rep=False,
        ),
        donate_argnums=tuple(range(n_params, nspecs)),
        keep_unused=True,
    )
    runner = (sharded, in_names, out_names, out_avals, zero_shapes)
    _RUNNER_CACHE[id(nc)] = runner
    return runner


def _run_spmd(nc, in_maps):
    sharded, in_names, out_names, out_avals, zero_shapes = _get_runner(nc)
    concat_in = [
        np.concatenate([np.asarray(in_maps[c][name]) for c in range(NCORES)], axis=0)
        for name in in_names
    ]
    concat_zeros = [
        np.zeros((NCORES * s[0], *s[1:]), dt) for (s, dt) in zero_shapes
    ]
    import time as _time

    t0 = _time.perf_counter()
    out_arrs = sharded(*concat_in, *concat_zeros)
    out_arrs = [np.asarray(a) for a in out_arrs]
    _run_spmd.last_wall_s = _time.perf_counter() - t0
    return [
        {
            name: out_arrs[i].reshape(NCORES, *out_avals[i].shape)[c]
            for i, name in enumerate(out_names)
        }
        for c in range(NCORES)
    ]


_run_spmd.last_wall_s = None

# channel permutation: reference gate order [i,f,g,o] -> kernel order [g,f,i,o]
_PERM = np.concatenate([
    np.arange(2 * H, 3 * H),  # g
    np.arange(1 * H, 2 * H),  # f
    np.arange(0 * H, 1 * H),  # i
    np.arange(3 * H, 4 * H),  # o
])


def _prep_direction_weights(W_ih, W_hh, b_ih, b_hh):
    wih = np.ascontiguousarray(W_ih[_PERM].T).astype(np.float32)  # [D, G]
    whh = np.ascontiguousarray(W_hh[_PERM].T).astype(np.float32)  # [H, G]
    bsum = (b_ih + b_hh).astype(np.float32)[_PERM]  # [G]
    # tanh(z) = 2*sigmoid(2z) - 1 for the g-gate: fold the 2z into weights
    wih[:, 0:H] *= 2.0
    whh[:, 0:H] *= 2.0
    bsum[0:H] *= 2.0
    wih = wih.astype(np.float16)
    whh = whh.astype(F8_NP)
    maskvec = np.zeros(G, np.float32)
    maskvec[H:2 * H] = 1.0   # f block: +BIG on padded steps
    maskvec[2 * H:3 * H] = -1.0  # i block: -BIG on padded steps
    augw = np.stack([bsum, maskvec]).astype(np.float16)  # [2, G]
    return wih, whh, augw


def _prep_core_inputs(x, lengths, wih, whh, augw, q, reverse, T):
    xs = x[q * BC:(q + 1) * BC, :, :]  # [BC, T, D]
    ls = lengths[q * BC:(q + 1) * BC]  # [BC]
    if reverse:
        xs = xs[:, ::-1, :]
    xT = np.ascontiguousarray(xs.transpose(2, 1, 0).reshape(D, T * BC))
    mask = (ls[None, :] > np.arange(T)[:, None]).astype(np.float32)  # [T, BC]
    if reverse:
        mask = mask[::-1]
    mb = BIG * (1.0 - mask)  # [T, BC]
    aug = np.stack([np.ones(T * BC, np.float32), mb.reshape(T * BC)])
    return {
        "xT": xT.astype(np.float16),
        "wih": wih,
        "whh": whh,
        "augw": augw,
        "aug": aug.astype(np.float16),
    }


def _assemble_direction(houts, lengths, T, reverse):
    hs = []
    for q in range(4):
        h = np.asarray(houts[q]).astype(np.float32)  # [T, 128, 32]
        h = h.reshape(T, 128, KT, BC).transpose(3, 0, 2, 1).reshape(BC, T, H)
        hs.append(h)
    h = np.concatenate(hs, axis=0)  # [B, T, H]
    if reverse:
        h = h[:, ::-1, :]
    else:
        idx = np.minimum(np.arange(T)[None, :], (lengths - 1)[:, None])  # [B, T]
        h = h[np.arange(B)[:, None], idx]
    return h


def kernel(x, lengths, W_ih_f, W_hh_f, b_ih_f, b_hh_f, W_ih_b, W_hh_b, b_ih_b, b_hh_b):
    T = x.shape[1]
    x = np.asarray(x, dtype=np.float32)
    lengths = np.asarray(lengths).astype(np.int64)

    wf = _prep_direction_weights(W_ih_f, W_hh_f, b_ih_f, b_hh_f)
    wb = _prep_direction_weights(W_ih_b, W_hh_b, b_ih_b, b_hh_b)

    in_maps = []
    for r in range(NCORES):
        reverse = r >= 4
        q = r % 4
        m = _prep_core_inputs(x, lengths, *(wb if reverse else wf), q, reverse, T)
        in_maps.append(m)

    nc = _get_nc(T)
    results = _run_spmd(nc, in_maps)
    kernel.last_wall_s = _run_spmd.last_wall_s

    h_f = _assemble_direction(
        [results[r]["hout"] for r in range(4)], lengths, T, reverse=False
    )
    h_b = _assemble_direction(
        [results[r]["hout"] for r in range(4, 8)], lengths, T, reverse=True
    )
    return np.concatenate([h_f, h_b], axis=-1).astype(np.float32)


kernel.last_exec_time_ns = None
kernel.last_wall_s = None
